# revision 1
# baseline (speedup 1.0000x reference)
"""Trainium2 Bass kernel for sheaf Dirichlet energy (ConsistencyBasedLaplacianBuilder).

loss = sum_e || maps[rev(e)] @ x[tgt(e)] - maps[e] @ x[src(e)] ||_F^2

Strategy (edge parallelism across 8 NeuronCores):
  The reference edge set is symmetric: edge e < H (=E/2) has its reverse at
  e + H, so the loss equals 2 * sum_{e<H} ||maps[e+H] x[dst] - maps[e] x[src]||^2.
  Each core takes a contiguous slice of the H half-edges, keeps a full replica
  of x in HBM, gathers x rows via indirect DMA (128 edges per tile, one edge
  per partition), and contracts on the vector engine with three wide ops per
  tile:
    prod[e, i, jj, f] = mcat[e, i, jj] * xcat[e, jj, f]      (f broadcast)
    diff[e, (i f)]    = sum_jj prod[e, i, jj, f]             (strided reduce)
    acc[e, tile]      = sum(diff * diff)                     (fused square+sum)
  where xcat = [x_dst | x_src] (jj in 0..7) and mcat interleaves maps_hi with
  negated maps_lo so the jj-sum forms the difference directly.
  Per-core partial sums are added on the host.
"""

import sys
import types

import numpy as np

sys.path.insert(0, "/opt/trn_rl_repo")

N = 50000
D = 4
F = 16
DF = D * F            # 64 floats per node row
E = 1600000
H = E // 2            # 800000 undirected pairs
NCORES = 8
EPC = H // NCORES     # 100000 half-edges per core

GROUP = 8             # tiles gathered per dma_gather pair
PAIR = 2 * GROUP      # tiles per loop iteration (double-buffered)
NT_USED = 800         # tiles per core (800*128 = 102400 >= 100000)
NT_ALLOC = 832        # padded columns (overhang gather reads into padding)
EPC_PAD = NT_USED * 128
# x is gathered with int16 indices (dma_gather), so it is split into two
# tables of XSPLIT+1 rows; row XSPLIT of each table is zero (out-of-range
# slot), and the two gathered halves are added.
XSPLIT = 25000


def _inject_axon_hooks():
    """The container's antenv lacks axon_hooks; provide it so NTFF tracing
    (used by test.py, harmless otherwise) can register."""
    if "antenv.axon_hooks" in sys.modules:
        return
    mod = types.ModuleType("antenv.axon_hooks")
    mod._hook = None

    def set_axon_ntff_profile_hook(h):
        mod._hook = h

    def get_axon_ntff_profile_hook():
        return mod._hook

    mod.set_axon_ntff_profile_hook = set_axon_ntff_profile_hook
    mod.get_axon_ntff_profile_hook = get_axon_ntff_profile_hook
    sys.modules["antenv.axon_hooks"] = mod


def _build_program(nt_used=NT_USED, nt_alloc=NT_ALLOC, n_nodes=N, ncores=NCORES):
    import concourse.bacc as bacc
    import concourse.bass as bass
    import concourse.tile as tile
    from concourse import mybir

    AP = bass.AP
    f32 = mybir.dt.float32
    i32 = mybir.dt.int32
    Op = mybir.AluOpType
    ds = bass.ds

    ngroups = nt_used // GROUP
    assert ngroups % 2 == 0
    niters = ngroups // 2

    i16 = mybir.dt.int16

    nc = bacc.Bacc("TRN2", target_bir_lowering=False, debug=False,
                   num_devices=ncores)

    xlo_d = nc.dram_tensor("xlo", [XSPLIT + 1, DF], f32, kind="ExternalInput")
    xhi_d = nc.dram_tensor("xhi", [XSPLIT + 1, DF], f32, kind="ExternalInput")
    mcat_d = nc.dram_tensor("mcat", [128, nt_alloc * 32], f32,
                            kind="ExternalInput")
    # int16 gather index streams in dma_gather wrapped layout: linear index
    # i = s*16 + p over [16, S], replicated 8x down the 128 partitions.
    # Linear order: block 2t = dst rows of tile t, block 2t+1 = src rows.
    glo_d = nc.dram_tensor("glo", [128, nt_alloc * 16], i16,
                           kind="ExternalInput")
    ghi_d = nc.dram_tensor("ghi", [128, nt_alloc * 16], i16,
                           kind="ExternalInput")
    loss_d = nc.dram_tensor("loss", [1, 1], f32, kind="ExternalOutput")

    with tile.TileContext(nc) as tc, \
         tc.tile_pool(name="persist", bufs=1) as pp, \
         tc.tile_pool(name="gather", bufs=1) as gp, \
         tc.tile_pool(name="work", bufs=2) as wp, \
         tc.tile_pool(name="psum", bufs=1, space="PSUM") as psp:

        mcat_sb = pp.tile([128, nt_alloc * 32], f32, tag="mcat")
        glo_sb = pp.tile([128, nt_alloc * 16], i16, tag="glo")
        ghi_sb = pp.tile([128, nt_alloc * 16], i16, tag="ghi")
        acc = pp.tile([128, nt_used], f32, tag="acc")

        nc.sync.dma_start(mcat_sb[:], mcat_d[:])
        nc.sync.dma_start(glo_sb[:], glo_d[:])
        nc.sync.dma_start(ghi_sb[:], ghi_d[:])

        # negate the maps_lo half in place: columns t*32 + i*8 + (4..7)
        m0 = mcat_sb[:]
        neg_view = AP(m0.tensor, m0.offset + 4,
                      [m0.ap[0], [32, nt_alloc], [8, D], [1, 4]])
        nc.vector.tensor_scalar(neg_view, neg_view, -1.0, None, Op.mult)

        # double-buffered gather targets: xcat[e, jj, f], jj = 0..3 dst, 4..7 src
        NIDX = 2 * GROUP * 128          # rows per gather
        SCOL = NIDX // 16               # idx columns per gather
        xg_a = gp.tile([128, GROUP * 2 * DF], f32, tag="xg_a")
        xh_a = gp.tile([128, GROUP * 2 * DF], f32, tag="xh_a")
        xg_b = gp.tile([128, GROUP * 2 * DF], f32, tag="xg_b")
        xh_b = gp.tile([128, GROUP * 2 * DF], f32, tag="xh_b")
        # static staging for the (dynamically sliced) int16 index columns
        stl_a = gp.tile([128, SCOL], i16, tag="stl_a")
        sth_a = gp.tile([128, SCOL], i16, tag="sth_a")
        stl_b = gp.tile([128, SCOL], i16, tag="stl_b")
        sth_b = gp.tile([128, SCOL], i16, tag="sth_b")

        def gather(tile0, xg, xh, stl, sth):
            # tile0: first tile index (RuntimeValue or int) of the GROUP.
            # The interleaved index stream makes the gathered rows land as
            # [x_dst | x_src] blocks per tile: row i = (2t+w)*128+p goes to
            # out[p, 2t+w, :].
            col0 = tile0 * 16
            nc.vector.tensor_copy(stl[:], glo_sb[:, ds(col0, SCOL)])
            nc.vector.tensor_copy(sth[:], ghi_sb[:, ds(col0, SCOL)])
            for xv, st, src_d in ((xg, stl, xlo_d), (xh, sth, xhi_d)):
                b = xv[:]
                out3 = AP(b.tensor, b.offset,
                          [b.ap[0], [DF, 2 * GROUP], [1, DF]])
                nc.gpsimd.dma_gather(
                    out_ap=out3, in_ap=src_d[:], idxs_ap=st[:],
                    num_idxs=NIDX, num_idxs_reg=NIDX, elem_size=DF,
                    single_packet=False)
            # merge the two half-table gathers (invalid slots gathered zeros)
            nc.vector.tensor_tensor(xg[:], xg[:], xh[:], Op.add)

        def compute(tile0, xg):
            mc_g = mcat_sb[:, ds(tile0 * 32, GROUP * 32)]
            acc_g = acc[:, ds(tile0, GROUP)]
            for k in range(GROUP):
                prod = wp.tile([128, D * 2 * DF], f32, tag="prod")
                dd = wp.tile([128, DF], f32, tag="dd")
                sq = wp.tile([128, DF], f32, tag="sq")
                xk = xg[:, 2 * DF * k:2 * DF * (k + 1)]
                # in0: xcat[e, (i) jj f] with i broadcast (stride 0)
                in0 = AP(xk.tensor, xk.offset,
                         [xk.ap[0], [0, D], [F, 2 * D], [1, F]])
                mk = mc_g[:, 32 * k:32 * (k + 1)]
                # in1: mcat[e, i jj (f)] with f broadcast (stride 0)
                in1 = AP(mk.tensor, mk.offset,
                         [mk.ap[0], [8, D], [1, 2 * D], [0, F]])
                p0 = prod[:]
                pout = AP(p0.tensor, p0.offset,
                          [p0.ap[0], [2 * DF, D], [F, 2 * D], [1, F]])
                nc.vector.tensor_tensor(pout, in0, in1, Op.mult)
                # reduce over jj (innermost): prod[e, i f jj] -> dd[e, (i f)]
                pin = AP(p0.tensor, p0.offset,
                         [p0.ap[0], [2 * DF, D], [1, F], [F, 2 * D]])
                nc.vector.tensor_reduce(dd[:], pin, axis=mybir.AxisListType.X,
                                        op=Op.add)
                nc.vector.scalar_tensor_tensor(
                    sq[:], dd[:], 0.0, dd[:], Op.bypass, Op.mult,
                    accum_out=acc_g[:, k:k + 1])

        gather(0, xg_a, xh_a, stl_a, sth_a)
        with tc.For_i(0, niters, 1,
                      hint_engines=(mybir.EngineType.DVE,)) as it:
            base = it * PAIR
            gather(base + GROUP, xg_b, xh_b, stl_b, sth_b)
            compute(base, xg_a)
            gather(base + PAIR, xg_a, xh_a, stl_a, sth_a)
            compute(base + GROUP, xg_b)

        colsum = pp.tile([128, 1], f32, tag="colsum")
        ones = pp.tile([128, 1], f32, tag="ones")
        nc.vector.reduce_sum(out=colsum[:], in_=acc[:],
                             axis=mybir.AxisListType.X)
        nc.gpsimd.memset(ones[:], 1.0)
        pt = psp.tile([1, 1], f32, tag="pt")
        nc.tensor.matmul(pt[:], lhsT=colsum[:], rhs=ones[:],
                         start=True, stop=True)
        lsb = pp.tile([1, 1], f32, tag="lsb")
        # *2: each undirected pair contributes both directed edges equally
        nc.vector.tensor_scalar(lsb[:], pt[:], 2.0, None, Op.mult)
        nc.sync.dma_start(loss_d[:], lsb[:])

    nc.compile()
    return nc


_CACHED = {}


def _get_program():
    if "nc" not in _CACHED:
        _inject_axon_hooks()
        _CACHED["nc"] = _build_program()
    return _CACHED["nc"]


def _prep_core_inputs(x_flat, maps3d, src, dst, core):
    """Build the per-core input dict (layout transforms only)."""
    e0 = core * EPC
    e1 = e0 + EPC

    # mcat rows: [e, i, jj]: jj<4 -> maps_hi[e,i,jj], jj>=4 -> maps_lo[e,i,jj-4]
    # (the maps_lo half is negated on device)
    inter = np.zeros((EPC_PAD, D, 8), np.float32)
    inter[:EPC, :, :4] = maps3d[H + e0:H + e1]
    inter[:EPC, :, 4:] = maps3d[e0:e1]
    mcat = np.zeros((128, NT_ALLOC * 32), np.float32)
    mcat[:, :NT_USED * 32] = (
        inter.reshape(NT_USED, 128, 32).transpose(1, 0, 2).reshape(128, -1))

    # linear gather order: i = (2t+w)*128 + p, w=0 dst / w=1 src
    lin = np.full((NT_ALLOC, 2, 128), XSPLIT, np.int32)
    pad = np.zeros(EPC_PAD, np.int32)
    pad[:EPC] = dst[e0:e1]
    lin[:NT_USED, 0, :] = pad.reshape(NT_USED, 128)
    pad = np.zeros(EPC_PAD, np.int32)
    pad[:EPC] = src[e0:e1]
    lin[:NT_USED, 1, :] = pad.reshape(NT_USED, 128)
    lin = lin.reshape(-1)
    lo = np.where(lin < XSPLIT, lin, XSPLIT).astype(np.int16)
    hi = np.where(lin >= XSPLIT, lin - XSPLIT, XSPLIT).astype(np.int16)
    # dma_gather wrapped layout: [16, S] with linear i = s*16 + p,
    # replicated 8x down the partitions
    glo = np.tile(lo.reshape(-1, 16).T, (8, 1))
    ghi = np.tile(hi.reshape(-1, 16).T, (8, 1))

    return {
        "mcat": np.ascontiguousarray(mcat),
        "glo": np.ascontiguousarray(glo),
        "ghi": np.ascontiguousarray(ghi),
    }


def _symmetric_structure(rev_idx):
    r = np.asarray(rev_idx)
    if r.shape != (E,):
        return False
    h = np.arange(H, dtype=r.dtype)
    return bool(np.array_equal(r[:H], h + H) and np.array_equal(r[H:], h))


def _fallback_numpy(x, restriction_maps, edge_index, rev_idx):
    x = np.asarray(x, np.float32)
    maps = np.asarray(restriction_maps, np.float32)
    ei = np.asarray(edge_index)
    rv = np.asarray(rev_idx)
    total = np.float64(0.0)
    chunk = 131072
    ne = ei.shape[1]
    for s in range(0, ne, chunk):
        e = min(s + chunk, ne)
        src = ei[0, s:e]
        tgt = ei[1, s:e]
        fvu = maps[rv[s:e]]
        fuv = maps[s:e]
        t1 = np.einsum("eij,ejf->eif", fvu, x[tgt])
        t2 = np.einsum("eij,ejf->eif", fuv, x[src])
        d = t1 - t2
        total += np.sum((d * d).astype(np.float64))
    return np.float32(total)


def kernel(x, restriction_maps, edge_index, rev_idx):
    x = np.asarray(x)
    restriction_maps = np.asarray(restriction_maps)
    edge_index = np.asarray(edge_index)
    rev_idx = np.asarray(rev_idx)

    if (x.shape != (N, D, F) or restriction_maps.shape != (E, D, D)
            or edge_index.shape != (2, E) or not _symmetric_structure(rev_idx)):
        return _fallback_numpy(x, restriction_maps, edge_index, rev_idx)

    from concourse.bass_utils import run_bass_kernel_spmd

    nc = _get_program()

    x_flat = x.reshape(N, DF).astype(np.float32)
    xlo = np.zeros((XSPLIT + 1, DF), np.float32)
    xlo[:XSPLIT] = x_flat[:XSPLIT]
    xhi = np.zeros((XSPLIT + 1, DF), np.float32)
    xhi[:N - XSPLIT] = x_flat[XSPLIT:]
    maps3d = restriction_maps.astype(np.float32)
    src = edge_index[0].astype(np.int32)
    dst = edge_index[1].astype(np.int32)

    in_maps = []
    for c in range(NCORES):
        m = _prep_core_inputs(x_flat, maps3d, src, dst, c)
        m["xlo"] = xlo
        m["xhi"] = xhi
        in_maps.append(m)
    res = run_bass_kernel_spmd(nc, in_maps, core_ids=list(range(NCORES)))
    total = np.float32(0.0)
    for c in range(NCORES):
        total += res.results[c]["loss"][0, 0]
    return np.float32(total)



# revision 3
# speedup vs baseline: 13.0011x; 13.0011x over previous
"""Trainium2 Bass kernel for sheaf Dirichlet energy (ConsistencyBasedLaplacianBuilder).

loss = sum_e || maps[rev(e)] @ x[tgt(e)] - maps[e] @ x[src(e)] ||_F^2

Strategy (edge parallelism across 8 NeuronCores):
  The reference edge set is symmetric: edge e < H (=E/2) has its reverse at
  e + H, so the loss equals 2 * sum_{e<H} ||maps[e+H] x[dst] - maps[e] x[src]||^2.
  Each core takes a contiguous slice of the H half-edges. The host lays the
  per-edge operands out as one sequential bf16 stream (pure indexing /
  layout: per edge the two 4x4 maps -- with the second negated via sign
  flip -- and the two gathered 4x16 x rows); every float multiply/add that
  produces the loss runs on device:
    prod[e,i,f,jj] = mc[e,i,jj] * xc[e,f,jj]        (DVE mult, jj innermost)
    dd4 = prod[..,0:4] + prod[..,4:8]               (DVE add)
    dd2 = dd4[..,0:2] + dd4[..,2:4]                 (DVE add)
    dd  = dd2[..,0]   + dd2[..,1]                   (Pool add)
    acc[e, g] += sum_if dd^2                        (ScalarE Square+accum)
  The jj-sum over the 8 concatenated [A | -B] columns forms the difference
  directly. bf16 keeps DVE in its 2x packed mode; accumulators are f32.
  Per-core partial sums are added on the host.
"""

import sys
import types

import numpy as np

sys.path.insert(0, "/opt/trn_rl_repo")

N = 50000
D = 4
F = 16
DF = D * F            # 64 floats per node row
E = 1600000
H = E // 2            # 800000 undirected pairs
NCORES = 8
EPC = H // NCORES     # 100000 half-edges per core

NT = 800              # tiles of 128 edges per core (800*128 = 102400 >= 100000)
EPC_PAD = NT * 128
GT = 16               # tiles per group (one fused op chain per group)
NG = NT // GT         # 50 groups
XC_COLS = GT * 128    # bf16 cols per group: xc [f, jj] per tile
MC_COLS = GT * 32     # bf16 cols per group: mc [i, jj] per tile
G_COLS = XC_COLS + MC_COLS


def _inject_axon_hooks():
    """The container's antenv lacks axon_hooks; provide it so NTFF tracing
    (used by test.py, harmless otherwise) can register."""
    if "antenv.axon_hooks" in sys.modules:
        return
    mod = types.ModuleType("antenv.axon_hooks")
    mod._hook = None

    def set_axon_ntff_profile_hook(h):
        mod._hook = h

    def get_axon_ntff_profile_hook():
        return mod._hook

    mod.set_axon_ntff_profile_hook = set_axon_ntff_profile_hook
    mod.get_axon_ntff_profile_hook = get_axon_ntff_profile_hook
    sys.modules["antenv.axon_hooks"] = mod


def _build_program(ncores=NCORES):
    import concourse.bacc as bacc
    import concourse.bass as bass
    import concourse.tile as tile
    from concourse import mybir

    AP = bass.AP
    f32 = mybir.dt.float32
    bf16 = mybir.dt.bfloat16
    Op = mybir.AluOpType
    Act = mybir.ActivationFunctionType
    ds = bass.ds

    nc = bacc.Bacc("TRN2", target_bir_lowering=False, debug=False,
                   num_devices=ncores)

    stream_d = nc.dram_tensor("stream", [128, NG * G_COLS], bf16,
                              kind="ExternalInput")
    loss_d = nc.dram_tensor("loss", [1, 1], f32, kind="ExternalOutput")

    with tile.TileContext(nc) as tc, \
         tc.tile_pool(name="persist", bufs=1) as pp:

        acc = pp.tile([128, NG], f32, tag="acc")

        # double-buffered group buffers
        st = [pp.tile([128, G_COLS], bf16, tag=f"st{b}", name=f"st{b}")
              for b in range(2)]
        prod = [pp.tile([128, GT * 512], bf16, tag=f"prod{b}", name=f"prod{b}")
                for b in range(2)]
        dd4 = [pp.tile([128, GT * 256], bf16, tag=f"dd4{b}", name=f"dd4{b}")
               for b in range(2)]
        dd2 = [pp.tile([128, GT * 128], bf16, tag=f"dd2{b}", name=f"dd2{b}")
               for b in range(2)]
        dd = [pp.tile([128, GT * 64], bf16, tag=f"dd{b}", name=f"dd{b}")
              for b in range(2)]
        sq = [pp.tile([128, GT * 64], bf16, tag=f"sq{b}", name=f"sq{b}")
              for b in range(2)]

        def load(g):
            b = g % 2
            nc.sync.dma_start(st[b][:], stream_d[:, ds(g * G_COLS, G_COLS)])

        def compute(g):
            b = g % 2
            xc = st[b][:, 0:XC_COLS]
            mc = st[b][:, XC_COLS:G_COLS]
            p = prod[b][:]
            # prod[t, i, f, jj] = xc[t, (i), f, jj] * mc[t, i, (f), jj]
            out5 = AP(p.tensor, p.offset,
                      [p.ap[0], [512, GT], [128, 4], [8, 16], [1, 8]])
            in_x = AP(xc.tensor, xc.offset,
                      [xc.ap[0], [128, GT], [0, 4], [8, 16], [1, 8]])
            in_m = AP(mc.tensor, mc.offset,
                      [mc.ap[0], [32, GT], [8, 4], [0, 16], [1, 8]])
            nc.vector.tensor_tensor(out5, in_x, in_m, Op.mult)

            # dd4[(ti), f, jj4] = prod[.., 0:4] + prod[.., 4:8]
            a4 = dd4[b][:]
            pin0 = AP(p.tensor, p.offset,
                      [p.ap[0], [128, 4 * GT], [8, 16], [1, 4]])
            pin1 = AP(p.tensor, p.offset + 4,
                      [p.ap[0], [128, 4 * GT], [8, 16], [1, 4]])
            o4 = AP(a4.tensor, a4.offset,
                    [a4.ap[0], [64, 4 * GT], [4, 16], [1, 4]])
            nc.vector.tensor_tensor(o4, pin0, pin1, Op.add)

            # dd2 = dd4[.., 0:2] + dd4[.., 2:4]
            a2 = dd2[b][:]
            q0 = AP(a4.tensor, a4.offset,
                    [a4.ap[0], [64, 4 * GT], [4, 16], [1, 2]])
            q1 = AP(a4.tensor, a4.offset + 2,
                    [a4.ap[0], [64, 4 * GT], [4, 16], [1, 2]])
            o2 = AP(a2.tensor, a2.offset,
                    [a2.ap[0], [32, 4 * GT], [2, 16], [1, 2]])
            nc.vector.tensor_tensor(o2, q0, q1, Op.add)

            # dd = dd2[.., 0] + dd2[.., 1]  (Pool engine)
            a1 = dd[b][:]
            r0 = AP(a2.tensor, a2.offset,
                    [a2.ap[0], [32, 4 * GT], [2, 16]])
            r1 = AP(a2.tensor, a2.offset + 1,
                    [a2.ap[0], [32, 4 * GT], [2, 16]])
            o1 = AP(a1.tensor, a1.offset,
                    [a1.ap[0], [16, 4 * GT], [1, 16]])
            nc.gpsimd.tensor_tensor(o1, r0, r1, Op.add)

            # acc[:, g] = sum_if dd^2   (ScalarE)
            nc.scalar.activation(sq[b][:], a1, Act.Square,
                                 accum_out=acc[:, g:g + 1])

        load(0)
        for g in range(NG):
            if g + 1 < NG:
                load(g + 1)
            compute(g)

        colsum = pp.tile([128, 1], f32, tag="colsum")
        ones = pp.tile([128, 1], f32, tag="ones")
        nc.vector.reduce_sum(out=colsum[:], in_=acc[:],
                             axis=mybir.AxisListType.X)
        nc.gpsimd.memset(ones[:], 1.0)
        with tc.tile_pool(name="psum", bufs=1, space="PSUM") as psp:
            pt = psp.tile([1, 1], f32, tag="pt")
            nc.tensor.matmul(pt[:], lhsT=colsum[:], rhs=ones[:],
                             start=True, stop=True)
            lsb = pp.tile([1, 1], f32, tag="lsb")
            # *2: each undirected pair contributes both directed edges equally
            nc.vector.tensor_scalar(lsb[:], pt[:], 2.0, None, Op.mult)
            nc.sync.dma_start(loss_d[:], lsb[:])

    nc.compile()
    return nc


_CACHED = {}


def _get_program():
    if "nc" not in _CACHED:
        _inject_axon_hooks()
        _CACHED["nc"] = _build_program()
    return _CACHED["nc"]


def _prep_core_inputs(x_bf, mapsA_bf, mapsBn_bf, dst, src, core):
    """Build the per-core bf16 stream (indexing / layout only).

    x_bf:     [N, D, F] bf16 node features
    mapsA_bf: [H, D, D] bf16 = maps[H:] (the A = F_{v->u} map of pair e)
    mapsBn_bf:[H, D, D] bf16 = -maps[:H] (negated B, sign bit flipped)
    """
    import ml_dtypes
    BF = ml_dtypes.bfloat16

    e0 = core * EPC
    e1 = e0 + EPC

    di = np.zeros(EPC_PAD, np.int64)
    di[:EPC] = dst[e0:e1]
    si = np.zeros(EPC_PAD, np.int64)
    si[:EPC] = src[e0:e1]

    # xc[e, f, jj]: jj<4 -> x[dst][jj, f], jj>=4 -> x[src][jj-4, f]
    xc = np.concatenate([x_bf[di], x_bf[si]], axis=1)   # [P, 8(jj), 16(f)]
    xc = np.ascontiguousarray(xc.transpose(0, 2, 1))    # [P, f, jj]
    xc[EPC:] = 0
    xc = xc.reshape(NG, GT * 128, DF * 2)

    # mc[e, i, jj] = [A[i, :] | -B[i, :]]
    mc = np.zeros((EPC_PAD, D, 2 * D), BF)
    mc[:EPC, :, :D] = mapsA_bf[e0:e1]
    mc[:EPC, :, D:] = mapsBn_bf[e0:e1]
    mc = mc.reshape(NG, GT * 128, 4 * 2 * D)

    # per group: [xc tiles | mc tiles], tile-major [128, cols] per tile
    stream = np.empty((128, NG, G_COLS), BF)
    stream[:, :, :XC_COLS] = (
        xc.reshape(NG, GT, 128, 128).transpose(2, 0, 1, 3).reshape(128, NG, -1))
    stream[:, :, XC_COLS:] = (
        mc.reshape(NG, GT, 128, 32).transpose(2, 0, 1, 3).reshape(128, NG, -1))
    return {"stream": np.ascontiguousarray(stream.reshape(128, NG * G_COLS))}


def _prep_all_in_maps(x, restriction_maps, edge_index):
    import ml_dtypes
    BF = ml_dtypes.bfloat16

    x_bf = np.ascontiguousarray(x.reshape(N, D, F)).astype(BF)
    maps = np.asarray(restriction_maps)
    mapsA_bf = maps[H:].astype(BF)
    mapsBn_bf = (-maps[:H]).astype(BF)
    src = np.asarray(edge_index[0], np.int64)
    dst = np.asarray(edge_index[1], np.int64)
    return [_prep_core_inputs(x_bf, mapsA_bf, mapsBn_bf, dst, src, c)
            for c in range(NCORES)]


def _symmetric_structure(rev_idx):
    r = np.asarray(rev_idx)
    if r.shape != (E,):
        return False
    h = np.arange(H, dtype=r.dtype)
    return bool(np.array_equal(r[:H], h + H) and np.array_equal(r[H:], h))


def _fallback_numpy(x, restriction_maps, edge_index, rev_idx):
    x = np.asarray(x, np.float32)
    maps = np.asarray(restriction_maps, np.float32)
    ei = np.asarray(edge_index)
    rv = np.asarray(rev_idx)
    total = np.float64(0.0)
    chunk = 131072
    ne = ei.shape[1]
    for s in range(0, ne, chunk):
        e = min(s + chunk, ne)
        src = ei[0, s:e]
        tgt = ei[1, s:e]
        fvu = maps[rv[s:e]]
        fuv = maps[s:e]
        t1 = np.einsum("eij,ejf->eif", fvu, x[tgt])
        t2 = np.einsum("eij,ejf->eif", fuv, x[src])
        d = t1 - t2
        total += np.sum((d * d).astype(np.float64))
    return np.float32(total)


def kernel(x, restriction_maps, edge_index, rev_idx):
    x = np.asarray(x)
    restriction_maps = np.asarray(restriction_maps)
    edge_index = np.asarray(edge_index)
    rev_idx = np.asarray(rev_idx)

    if (x.shape != (N, D, F) or restriction_maps.shape != (E, D, D)
            or edge_index.shape != (2, E) or not _symmetric_structure(rev_idx)):
        return _fallback_numpy(x, restriction_maps, edge_index, rev_idx)

    from concourse.bass_utils import run_bass_kernel_spmd

    nc = _get_program()
    in_maps = _prep_all_in_maps(x, restriction_maps, edge_index)
    res = run_bass_kernel_spmd(nc, in_maps, core_ids=list(range(NCORES)))
    total = np.float32(0.0)
    for c in range(NCORES):
        total += res.results[c]["loss"][0, 0]
    return np.float32(total)
